# revision 21
# baseline (speedup 1.0000x reference)
"""Causal GQA attention (B=2, T=2048, D=2048, QH=16, KVH=4, HD=128) on 8 TRN2 cores.

Sharding: DP-2 over batch x TP-4 over KV-head groups.
  core c -> batch c//4, kv head c%4, q heads 4*(c%4)..4*(c%4)+3.
Each core computes a partial (T, D) output (its heads' contribution through wo);
the host sums the 4 partials per batch and stacks the two batches.

Device dataflow (everything transposed; no on-device activation transposes):
  - host feeds xT = x[b].T                            (D, T)
  - qT/kT = W^T x computed directly in [hd, t] layout; all six projection
    targets interleave per 128-contraction chunk so the PE never starves on
    the startup DMA stream
  - RoPE via swap-permutation matmul (rot = R @ qT) + DVE mul/add
  - scores per PAIR of 128-key blocks into one 2-bank PSUM tile [128,2,512];
    ONE ACT exp instruction covers the pair (amortizes ACT init + sync so the
    ACT engine runs below the PE rate during attention)
  - causal: fully-masked column ranges never computed; the 128x128 diagonal
    triangle is masked by a DVE multiply
  - O^T[hd, q] += V_blk^T @ expS^T; den += ones @ expS^T (broadcast denom)
  - normalize deferred one head: DVE evac (fp32) -> reciprocal_approx_fast
    (single op, ~5x faster than exact reciprocal) -> DVE multiply
  - out[t, d] = sum_h (OT_h)^T @ wo_h accumulated in PAIRED psum tiles
    [128,2,512] (two 512-col output chunks per tile); evac fp16 + one DMA per
    pair -> 4x less output DMA traffic than fp32 singles
  - ~16 dummy matmuls at t=0 keep the PE busy so the HAM clock gate lifts the
    4/8 throttle ~3.5us in, instead of ~27us
All matmuls fp16 (1 cycle/row on the PE; fp32 PSUM accumulation).
"""
import numpy as np
from contextlib import ExitStack

import concourse.bacc as bacc
import concourse.tile as tile
import concourse.mybir as mybir
from concourse.bass_utils import run_bass_kernel_spmd

B, T, D = 2, 2048, 2048
QH, KVH = 16, 4
HD = D // QH            # 128
P = 128
NT = T // 512           # 4 t-tiles of 512
DC = D // P             # 16 contraction chunks
KB = T // P             # 16 key blocks
F32 = mybir.dt.float32
CDT = mybir.dt.float16          # compute dtype on the PE (1 cycle/row)
NPDT = np.float16
AF = mybir.ActivationFunctionType
SCALE = float(1.0 / np.sqrt(HD))

_cached = {}


def _build():
    nc = bacc.Bacc("TRN2", target_bir_lowering=False, debug=False)
    # pre-shuffled on host so every DMA line is >=4KB contiguous
    xT = nc.dram_tensor("xT", [NT, 4, P, 4, 512], CDT, kind="ExternalInput")
    wq = nc.dram_tensor("wq", [4, P, 4, 512], CDT, kind="ExternalInput")
    wk = nc.dram_tensor("wk", [P, DC, HD], CDT, kind="ExternalInput")
    wv = nc.dram_tensor("wv", [P, DC, HD], CDT, kind="ExternalInput")
    wo = nc.dram_tensor("wo", [4 * HD, D], CDT, kind="ExternalInput")
    cosT = nc.dram_tensor("cosT", [HD, T], CDT, kind="ExternalInput")
    ssinT = nc.dram_tensor("ssinT", [HD, T], CDT, kind="ExternalInput")
    rmat = nc.dram_tensor("rmat", [P, P], CDT, kind="ExternalInput")
    tri = nc.dram_tensor("tri", [P, P], CDT, kind="ExternalInput")
    ident = nc.dram_tensor("ident", [P, P], CDT, kind="ExternalInput")
    out = nc.dram_tensor("out", [T, D], CDT, kind="ExternalOutput")

    with tile.TileContext(nc) as tc, ExitStack() as ctx:
        const = ctx.enter_context(tc.tile_pool(name="const", bufs=1))
        kvres = ctx.enter_context(tc.tile_pool(name="kvres", bufs=1))
        xc_pool = ctx.enter_context(tc.tile_pool(name="xc", bufs=8))
        qr_pool = ctx.enter_context(tc.tile_pool(name="qr", bufs=8))
        tmp_pool = ctx.enter_context(tc.tile_pool(name="tmp", bufs=3))
        e_pool = ctx.enter_context(tc.tile_pool(name="ep", bufs=6))
        ot_pool = ctx.enter_context(tc.tile_pool(name="ot", bufs=2))
        oev_pool = ctx.enter_context(tc.tile_pool(name="oev", bufs=4))
        bc_pool = ctx.enter_context(tc.tile_pool(name="bc", bufs=2))

        # PSUM: 8 banks total.  pp = paired tiles (2 banks each, 2 bufs);
        # po/pd = single-bank tiles used for k/v/rot psums in phase A and the
        # O / denominator accumulators in phase B.
        pp = ctx.enter_context(tc.tile_pool(name="pp", bufs=2, space="PSUM"))
        po = ctx.enter_context(tc.tile_pool(name="po", bufs=2, space="PSUM"))
        pd = ctx.enter_context(tc.tile_pool(name="pd", bufs=2, space="PSUM"))

        # ---- resident constants ----
        wq_sb = const.tile([P, DC, 4 * HD], CDT, tag="wq")
        wk_sb = const.tile([P, DC, HD], CDT, tag="wk")
        wv_sb = const.tile([P, DC, HD], CDT, tag="wv")
        wo_sb = const.tile([P, 4, D], CDT, tag="wo")
        cos_sb = const.tile([P, T], CDT, tag="cos")
        sin_sb = const.tile([P, T], CDT, tag="sin")
        rm_sb = const.tile([P, P], CDT, tag="rm")
        tri_sb = const.tile([P, P], CDT, tag="tri")
        id_sb = const.tile([P, P], CDT, tag="id")
        ones_mat = const.tile([P, P], CDT, tag="ones")
        dum_sb = const.tile([P, 256], CDT, tag="dum")

        kT_all = kvres.tile([P, T], CDT, tag="kT")
        v_all = kvres.tile([P, KB, HD], CDT, tag="V")

        wo_v = wo.rearrange("(c p) n -> c p n", p=P)

        # ---- HAM warmup: keep the PE busy from t=0 so the clock gate lifts
        # at ~3.5us.  Dummies read a memset tile, write a scratch psum.
        nc.vector.memset(dum_sb[:], 0.0)
        dum_ps = pp.tile([P, 2, 512], F32, tag="pair", name="dum_ps")
        for i in range(16):
            nc.tensor.matmul(dum_ps[:, 0, 0:256], dum_sb[:, 0:128],
                             dum_sb[:, 0:256], start=True, stop=True,
                             skip_group_check=True)

        # ---- startup DMAs for tile 0, paced per-chunk ----
        # First 4 contraction chunks arrive individually (x, wq, wk, wv per
        # chunk) so the 6-way interleaved projection can start ~1.2us in;
        # later groups arrive 4-chunks at a time.
        xg0 = []
        for g in range(4):
            xg = xc_pool.tile([P, 4, 512], CDT, tag="xc", name=f"xc0_{g}")
            xg0.append(xg)
        for dc in range(4):
            nc.sync.dma_start(out=xg0[0][:, dc, :], in_=xT[0, 0, :, dc, :])
            nc.gpsimd.dma_start(out=wq_sb[:, dc, :], in_=wq[0, :, dc, :])
        nc.vector.memset(ones_mat[:], 1.0)
        for g in range(1, 4):
            if g == 3:
                nc.sync.dma_start(out=wk_sb[:], in_=wk[:])
            nc.sync.dma_start(out=xg0[g][:], in_=xT[0, g])
            nc.gpsimd.dma_start(out=wq_sb[:, g * 4:(g + 1) * 4, :], in_=wq[g])
            if g == 1:
                nc.sync.dma_start(out=rm_sb[:], in_=rmat[:])
                nc.sync.dma_start(out=tri_sb[:], in_=tri[:])
                nc.sync.dma_start(out=id_sb[:], in_=ident[:])
        nc.gpsimd.dma_start(out=cos_sb[:], in_=cosT[:])
        nc.gpsimd.dma_start(out=sin_sb[:], in_=ssinT[:])
        nc.gpsimd.dma_start(out=wv_sb[:], in_=wv[:])

        def load_x_tile(tt):
            grps = []
            for g in range(4):
                xg = xc_pool.tile([P, 4, 512], CDT, tag="xc", name=f"xc{tt}_{g}")
                nc.gpsimd.dma_start(out=xg[:], in_=xT[tt, g])
                grps.append(xg)
            return grps

        def rope(dst_ap, src_ps, tt, nm, pool=None, tag="o", dve_evac=False):
            """dst[hd, 512] = src*cos + (R@src)*ssin.  src is PSUM."""
            pool = pool if pool is not None else po
            c_sl = cos_sb[:, tt * 512:(tt + 1) * 512]
            s_sl = sin_sb[:, tt * 512:(tt + 1) * 512]
            sb = tmp_pool.tile([P, 512], CDT, tag="evac", name=f"ev_{nm}")
            if dve_evac:
                with nc.allow_low_precision(reason="evac"):
                    nc.vector.tensor_copy(sb[:], src_ps[:])
            else:
                nc.scalar.copy(sb[:], src_ps[:])
            rot_ps = pool.tile([P, 512], F32, tag=tag, name=f"rot_{nm}")
            nc.tensor.matmul(rot_ps[:], rm_sb[:], sb[:], start=True, stop=True)
            t1 = tmp_pool.tile([P, 512], F32, tag="t1", name=f"t1_{nm}")
            nc.vector.tensor_mul(t1[:], sb[:], c_sl)
            t2 = tmp_pool.tile([P, 512], F32, tag="t2", name=f"t2_{nm}")
            nc.vector.tensor_mul(t2[:], rot_ps[:], s_sl)
            with nc.allow_low_precision(reason="fp16 store for PE"):
                nc.vector.tensor_add(dst_ap, t1[:], t2[:])

        def v_section(tt, vps):
            vt_sb = tmp_pool.tile([P, 512], CDT, tag="evac", name=f"vt{tt}")
            nc.scalar.copy(vt_sb[:], vps[:])
            tr_ps = pd.tile([P, 512], CDT, tag="d", name=f"vtr{tt}")
            for i in range(4):
                nc.tensor.transpose(tr_ps[:, i * P:(i + 1) * P],
                                    vt_sb[:, i * P:(i + 1) * P], id_sb[:])
            for i in range(4):
                with nc.allow_low_precision(reason="fp16 store"):
                    nc.vector.tensor_copy(v_all[:, tt * 4 + i, :],
                                          tr_ps[:, i * P:(i + 1) * P])

        def proj_phase(tt, qT_roped, xgs):
            tsl = slice(tt * 512, (tt + 1) * 512)
            q01 = pp.tile([P, 2, 512], F32, tag="pair", name=f"p{tt}q01")
            q23 = pp.tile([P, 2, 512], F32, tag="pair", name=f"p{tt}q23")
            # q chains first (at startup only x+wq are in early DMA stream;
            # later tiles have x resident, order keeps seams tight)
            nq = 4 if tt == 0 else 2
            for dc in range(DC):
                xc = xgs[dc // 4][:, dc % 4, :]
                st, sp = (dc == 0), (dc == DC - 1)
                for idx in range(nq):
                    nc.tensor.matmul(q01[:, idx, :] if idx < 2
                                     else q23[:, idx - 2, :],
                                     wq_sb[:, dc, idx * HD:(idx + 1) * HD],
                                     xc, start=st, stop=sp)
            sb2 = sb3 = None
            if tt == 0:
                sb2 = tmp_pool.tile([P, 512], CDT, tag="lev", bufs=4,
                                    name=f"lev{tt}_2")
                sb3 = tmp_pool.tile([P, 512], CDT, tag="lev", bufs=4,
                                    name=f"lev{tt}_3")
                with nc.allow_low_precision(reason="evac"):
                    nc.vector.tensor_copy(sb2[:], q23[:, 0, :])
                    nc.vector.tensor_copy(sb3[:], q23[:, 1, :])
            rope(qT_roped[0][:], q01[:, 0, :], tt, f"q{tt}_0")
            rope(qT_roped[1][:], q01[:, 1, :], tt, f"q{tt}_1")
            kps = po.tile([P, 512], F32, tag="o", name=f"p{tt}k")
            vps = pd.tile([P, 512], F32, tag="d", name=f"p{tt}v")
            for dc in range(DC):
                xc = xgs[dc // 4][:, dc % 4, :]
                st, sp = (dc == 0), (dc == DC - 1)
                for idx in range(2, 2 if tt == 0 else 4):
                    nc.tensor.matmul(q23[:, idx - 2, :],
                                     wq_sb[:, dc, idx * HD:(idx + 1) * HD],
                                     xc, start=st, stop=sp)
                nc.tensor.matmul(kps[:], wk_sb[:, dc, :], xc, start=st, stop=sp)
                nc.tensor.matmul(vps[:], wv_sb[:, dc, :], xc, start=st, stop=sp)
            rope(kT_all[:, tsl], kps, tt, f"k{tt}")
            v_section(tt, vps)
            if tt > 0:
                sb2 = tmp_pool.tile([P, 512], CDT, tag="lev", bufs=4,
                                    name=f"lev{tt}_2")
                sb3 = tmp_pool.tile([P, 512], CDT, tag="lev", bufs=4,
                                    name=f"lev{tt}_3")
                with nc.allow_low_precision(reason="evac"):
                    nc.vector.tensor_copy(sb2[:], q23[:, 0, :])
                    nc.vector.tensor_copy(sb3[:], q23[:, 1, :])

            def late_ropes():
                if tt == 0:
                    # feed the HAM activity window through the tile-0 seam;
                    # bare weight loads keep the array busy without writing
                    for _ in range(8):
                        nc.tensor.ldweights(weights=dum_sb[:, 0:128])
                # rotation + rope-muls for q2/q3, emitted inside phase B
                # (after head 0) so the seam stays tight.
                # Slots: q2 -> pd (after tr_ps), q3 -> po (after rot_k).
                for qi, sbx, pool, tg in ((2, sb2, pd, "d"), (3, sb3, po, "o")):
                    c_sl = cos_sb[:, tt * 512:(tt + 1) * 512]
                    s_sl = sin_sb[:, tt * 512:(tt + 1) * 512]
                    rot_ps = pool.tile([P, 512], F32, tag=tg,
                                       name=f"rot_l{tt}_{qi}")
                    nc.tensor.matmul(rot_ps[:], rm_sb[:], sbx[:],
                                     start=True, stop=True)
                    t1 = tmp_pool.tile([P, 512], F32, tag="t1",
                                       name=f"t1_l{tt}_{qi}")
                    nc.vector.tensor_mul(t1[:], sbx[:], c_sl)
                    t2 = tmp_pool.tile([P, 512], F32, tag="t2",
                                       name=f"t2_l{tt}_{qi}")
                    nc.vector.tensor_mul(t2[:], rot_ps[:], s_sl)
                    with nc.allow_low_precision(reason="fp16 store"):
                        nc.vector.tensor_add(qT_roped[qi][:], t1[:], t2[:])
            return late_ropes

        def finish_head(den_ps, o_ps, hh, tt, ot_sb, act_evac=False):
            bc_sb = bc_pool.tile([P, 512], F32, tag="bc", name=f"bs{tt}_{hh}")
            if act_evac:
                nc.scalar.copy(bc_sb[:], den_ps[:])
            else:
                nc.vector.tensor_copy(bc_sb[:], den_ps[:])
            rb_sb = bc_pool.tile([P, 512], F32, tag="rb", name=f"rb{tt}_{hh}")
            nc.vector.reciprocal_approx_fast(out=rb_sb[:], in_=bc_sb[:])
            for ch in range(2):
                cs = slice(ch * 256, (ch + 1) * 256)
                with nc.allow_low_precision(reason="norm"):
                    nc.vector.tensor_mul(ot_sb[:, hh, cs], o_ps[:, cs],
                                         rb_sb[:, cs])

        qT_next = [qr_pool.tile([P, 512], CDT, tag="qr", name=f"qr0_{i}")
                   for i in range(4)]
        late_next = proj_phase(0, qT_next, xg0)

        for tt in range(NT):
            tsl = slice(tt * 512, (tt + 1) * 512)
            qT_roped = qT_next
            late_ropes = late_next
            if tt > 0:
                late_ropes = proj_phase(tt, qT_roped, xgs_next)
            if tt + 1 < NT:
                # prefetch next x tile now; transfers overlap phase B/C
                xgs_next = load_x_tile(tt + 1)
                qT_next = [qr_pool.tile([P, 512], CDT, tag="qr",
                                        name=f"qr{tt + 1}_{i}") for i in range(4)]

            # ---------- Phase B: attention ----------
            npair = 2 * (tt + 1)
            ot_sb = ot_pool.tile([P, 4, 512], CDT, tag="ot", name=f"ot{tt}")
            pending_norm = []

            def emit_av_den(pk0, pl0, pk1, pl1, pe, pes, last, _s=None):
                o_ps, den_ps = _s
                nc.tensor.matmul(o_ps[:, pl0:512], v_all[:, pk0, :],
                                 pe[:, 0, pl0:512], start=(pk0 == 0), stop=False)
                nc.tensor.matmul(o_ps[:, pl1:512], v_all[:, pk1, :],
                                 pe[:, 1, pl1:512], start=False, stop=last)
                if pes is not None:
                    nc.tensor.matmul(den_ps[:], ones_mat, pes[:],
                                     start=(pk0 == 0), stop=last)
                else:
                    nc.tensor.matmul(den_ps[:, pl0:512], ones_mat,
                                     pe[:, 0, pl0:512],
                                     start=(pk0 == 0), stop=False)
                    nc.tensor.matmul(den_ps[:, pl1:512], ones_mat,
                                     pe[:, 1, pl1:512],
                                     start=False, stop=last)

            for hh in range(4):
                o_ps = po.tile([P, 512], F32, tag="o", name=f"o{tt}_{hh}")
                den_ps = pd.tile([P, 512], F32, tag="d", name=f"d{tt}_{hh}")
                emit = lambda *a, **k: emit_av_den(*a, **k, _s=(o_ps, den_ps))
                prev = None   # (kb0, lo0, kb1, lo1, e2, esum)
                for pj in range(npair):
                    kb0, kb1 = 2 * pj, 2 * pj + 1
                    d0, d1 = kb0 - 4 * tt, kb1 - 4 * tt
                    lo0 = d0 * P if d0 > 0 else 0
                    lo1 = d1 * P if d1 > 0 else 0
                    ps2 = pp.tile([P, 2, 512], F32, tag="pair",
                                  name=f"s{tt}_{hh}_{pj}")
                    nc.tensor.matmul(ps2[:, 0, lo0:512],
                                     kT_all[:, kb0 * P:(kb0 + 1) * P],
                                     qT_roped[hh][:, lo0:512],
                                     start=True, stop=True)
                    nc.tensor.matmul(ps2[:, 1, lo1:512],
                                     kT_all[:, kb1 * P:(kb1 + 1) * P],
                                     qT_roped[hh][:, lo1:512],
                                     start=True, stop=True)
                    e2 = e_pool.tile([P, 2, 512], CDT, tag="e",
                                     name=f"e{tt}_{hh}_{pj}")
                    if lo0 == lo1:
                        nc.scalar.activation(e2[:, :, lo0:512], ps2[:, :, lo0:512],
                                             AF.Exp, scale=SCALE)
                    else:
                        # diagonal pair: shared range in one op, plus the
                        # first block's extra 128-wide strip
                        nc.scalar.activation(e2[:, :, lo1:512], ps2[:, :, lo1:512],
                                             AF.Exp, scale=SCALE)
                        nc.scalar.activation(e2[:, 0, lo0:lo1], ps2[:, 0, lo0:lo1],
                                             AF.Exp, scale=SCALE)
                    if d0 >= 0:
                        with nc.allow_low_precision(reason="mask mult"):
                            nc.vector.tensor_mul(e2[:, 0, d0 * P:(d0 + 1) * P],
                                                 e2[:, 0, d0 * P:(d0 + 1) * P],
                                                 tri_sb[:])
                    if d1 >= 0:
                        with nc.allow_low_precision(reason="mask mult"):
                            nc.vector.tensor_mul(e2[:, 1, d1 * P:(d1 + 1) * P],
                                                 e2[:, 1, d1 * P:(d1 + 1) * P],
                                                 tri_sb[:])
                    if lo0 == 0 and lo1 == 0:
                        # off-diagonal pair: pre-sum the two blocks on DVE so
                        # ONE den matmul covers the pair
                        esum = e_pool.tile([P, 512], CDT, tag="es",
                                           name=f"es{tt}_{hh}_{pj}")
                        with nc.allow_low_precision(reason="den pre-sum"):
                            nc.vector.tensor_add(esum[:], e2[:, 0, :], e2[:, 1, :])
                    else:
                        esum = None
                    if prev is not None:
                        emit(*prev, last=False)
                    prev = (kb0, lo0, kb1, lo1, e2, esum)
                emit(*prev, last=True)
                if hh == 0:
                    late_ropes()
                # defer evac+recip+norm one head so the PE never waits
                if pending_norm:
                    finish_head(*pending_norm.pop(), tt, ot_sb)
                pending_norm.append((den_ps, o_ps, hh))
            last_norm = pending_norm.pop()

            if tt == 0:
                for c in range(4):
                    nc.sync.dma_start(out=wo_sb[:, c, :], in_=wo_v[c])

            # ---------- Phase C: output projection, paired psum tiles ----------
            # pair p covers output cols [d0*512, (d0+1)*512) and the next 512.
            # First two pairs emit heads 0-2, then the deferred head-3
            # normalization, then backfill head 3 and evacuate.
            lead = []
            for tc4 in range(4):
                trow = tt * 512 + tc4 * P
                for dp in range(2):
                    f2 = pp.tile([P, 2, 512], F32, tag="pair",
                                 name=f"f{tt}_{tc4}_{dp}")
                    nheads = 3 if len(lead) < 2 else 4
                    for hh in range(nheads):
                        for half in range(2):
                            doc = dp * 2 + half
                            nc.tensor.matmul(f2[:, half, :],
                                             ot_sb[:, hh, tc4 * P:(tc4 + 1) * P],
                                             wo_sb[:, hh, doc * 512:(doc + 1) * 512],
                                             start=(hh == 0),
                                             stop=(hh == 3))
                    if nheads == 3:
                        lead.append((f2, tc4, dp, trow))
                        if len(lead) == 2:
                            finish_head(*last_norm, tt, ot_sb, act_evac=True)
                            for lf2, ltc4, ldp, ltrow in lead:
                                for half in range(2):
                                    ldoc = ldp * 2 + half
                                    nc.tensor.matmul(
                                        lf2[:, half, :],
                                        ot_sb[:, 3, ltc4 * P:(ltc4 + 1) * P],
                                        wo_sb[:, 3, ldoc * 512:(ldoc + 1) * 512],
                                        start=False, stop=True)
                                o_ev = oev_pool.tile([P, 2, 512], CDT, tag="oev",
                                                     name=f"oe{tt}_{ltc4}_{ldp}")
                                with nc.allow_low_precision(reason="fp16 out"):
                                    if ldp % 2 == 0:
                                        nc.vector.tensor_copy(o_ev[:], lf2[:])
                                    else:
                                        nc.scalar.copy(o_ev[:], lf2[:])
                                nc.sync.dma_start(
                                    out=out[ltrow:ltrow + P,
                                            ldp * 1024:(ldp + 1) * 1024],
                                    in_=o_ev[:])
                        continue
                    o_ev = oev_pool.tile([P, 2, 512], CDT, tag="oev",
                                         name=f"oe{tt}_{tc4}_{dp}")
                    if tt == NT - 1 and tc4 == 3:
                        # tail: halves on both engines + eager DMAs
                        with nc.allow_low_precision(reason="fp16 out"):
                            nc.vector.tensor_copy(o_ev[:, 0, :], f2[:, 0, :])
                            nc.scalar.copy(o_ev[:, 1, :], f2[:, 1, :])
                        nc.sync.dma_start(out=out[trow:trow + P,
                                                  dp * 1024:dp * 1024 + 512],
                                          in_=o_ev[:, 0, :])
                        nc.sync.dma_start(out=out[trow:trow + P,
                                                  dp * 1024 + 512:(dp + 1) * 1024],
                                          in_=o_ev[:, 1, :])
                        continue
                    with nc.allow_low_precision(reason="fp16 out"):
                        if dp % 2 == 0:
                            nc.vector.tensor_copy(o_ev[:], f2[:])
                        else:
                            nc.scalar.copy(o_ev[:], f2[:])
                    nc.sync.dma_start(out=out[trow:trow + P,
                                              dp * 1024:(dp + 1) * 1024],
                                      in_=o_ev[:])
    nc.compile()
    return nc


def _host_tables():
    freqs = (1.0 / (np.float32(10000.0) **
                    (np.arange(0, HD, 2, dtype=np.float32) / np.float32(HD)))).astype(np.float32)
    t = np.arange(T, dtype=np.float32)
    ang = t[:, None] * freqs[None, :]
    cos = np.tile(np.cos(ang), (1, 2)).astype(np.float32)   # (T, HD)
    sin = np.tile(np.sin(ang), (1, 2)).astype(np.float32)
    cosT = np.ascontiguousarray(cos.T)                       # (HD, T)
    sinT = np.ascontiguousarray(sin.T)
    ssinT = sinT.copy()
    ssinT[:HD // 2] *= -1.0                                  # sign-folded sin
    rmat = np.zeros((P, P), dtype=np.float32)
    for j in range(HD // 2):
        rmat[j + HD // 2, j] = 1.0
    for j in range(HD // 2, HD):
        rmat[j - HD // 2, j] = 1.0
    tri = (np.arange(P)[:, None] <= np.arange(P)[None, :]).astype(np.float32)
    ident = np.eye(P, dtype=np.float32)
    return cosT, ssinT, rmat, tri, ident


def _make_in_maps(x, wq, wk, wv, wo):
    cosT, ssinT, rmat, tri, ident = _host_tables()
    x = np.asarray(x, dtype=np.float32)
    wq = np.asarray(wq, dtype=np.float32)
    wk = np.asarray(wk, dtype=np.float32)
    wv = np.asarray(wv, dtype=np.float32)
    wo = np.asarray(wo, dtype=np.float32)

    in_maps = []
    for c in range(8):
        b, h = divmod(c, 4)
        xTb = x[b].T.reshape(4, 4, P, NT, 512)          # (g, dc, p, tt, t)
        xS = np.ascontiguousarray(xTb.transpose(3, 0, 2, 1, 4))  # (tt,g,p,dc,t)
        wqS = np.ascontiguousarray(
            wq[:, h * 512:(h + 1) * 512].reshape(4, 4, P, 512).transpose(0, 2, 1, 3))
        wkS = np.ascontiguousarray(
            wk[:, h * HD:(h + 1) * HD].reshape(DC, P, HD).transpose(1, 0, 2))
        wvS = np.ascontiguousarray(
            wv[:, h * HD:(h + 1) * HD].reshape(DC, P, HD).transpose(1, 0, 2))
        in_maps.append({
            "xT": xS.astype(NPDT),
            "wq": wqS.astype(NPDT),
            "wk": wkS.astype(NPDT),
            "wv": wvS.astype(NPDT),
            "wo": np.ascontiguousarray(wo[h * 512:(h + 1) * 512, :]).astype(NPDT),
            "cosT": cosT.astype(NPDT), "ssinT": ssinT.astype(NPDT),
            "rmat": rmat.astype(NPDT), "tri": tri.astype(NPDT),
            "ident": ident.astype(NPDT),
        })
    return in_maps


def kernel(x, wq, wk, wv, wo):
    if "nc" not in _cached:
        _cached["nc"] = _build()
    nc = _cached["nc"]
    in_maps = _make_in_maps(x, wq, wk, wv, wo)
    try:
        res = run_bass_kernel_spmd(nc, in_maps, core_ids=list(range(8)))
    except Exception:
        # transient NRT/device hiccups recover on a clean retry
        res = run_bass_kernel_spmd(nc, in_maps, core_ids=list(range(8)))
    outs = [res.results[c]["out"].astype(np.float32) for c in range(8)]
    full = np.stack([outs[0] + outs[1] + outs[2] + outs[3],
                     outs[4] + outs[5] + outs[6] + outs[7]], axis=0)
    return full.astype(np.float32)
